# revision 1
# baseline (speedup 1.0000x reference)
"""Bass/Trainium2 kernel for GRAMAttention (B=2, T=2048, D=1024, H=16, hd=64).

Sharding: 8 cores = 2 batches (data parallel) x 4 head-groups of 4 heads
(tensor parallel: column-split wq/wk/wv, row-split wo). Each core computes a
partial (T, D) output for its batch; host sums the 4 head-group partials.

Per-core device pipeline (f32 data, f32r matmuls):
  host-transposed x^T (contiguous batched bf16 DMA) -> qT/kT projections
  into PSUM -> RoPE fused on DVE (stream_shuffle pair-swap + mul/mul/add
  reading PSUM directly) -> s^T = k @ q^T, both halves of a head pair into
  one 2-bank PSUM tile -> one wide exp on ACT per kt step -> causal mask
  post-exp via DVE multiply with precomputed 0/1 bf16 masks (diagonal tiles
  only) -> PV matmul with ones-column-augmented V (softmax denominator for
  free) -> normalize via DVE reciprocal (partition-shifted to lane 0) +
  Pool partition_broadcast -> row-split wo matmul.

Emission is software-pipelined at two levels: within an attention stream
pair the two halves' s-matmuls run one kt-step ahead of their PV
accumulations (so PE is never queued behind an exp it must wait for), and
PE-heavy filler chunks -- out_proj(j-1) and proj(j+1) -- are interleaved
between kt-steps because the attention phase is ACT(exp)-paced.
"""

import sys

if "/opt/trn_rl_repo" not in sys.path:
    sys.path.insert(0, "/opt/trn_rl_repo")

import ml_dtypes
import numpy as np

import concourse.bass as bass
import concourse.tile as tile
from concourse import bacc, mybir
from concourse.bass_utils import run_bass_kernel_spmd

B, T, D = 2, 2048, 1024
H = 16          # total heads
HPC = 4         # heads per core
HD = 64         # head dim
HG = H // HPC   # head groups (4) -> 8 cores = B * HG
DK = D // 128   # 8 contraction tiles for projections
NJ = T // 512   # 4 q/n slices
NT = T // 128   # 16 t tiles

F32 = mybir.dt.float32
# float32r: TF32-like relaxed fp32 matmul, 4x faster than fp32 at N>=256.
R = mybir.dt.float32r
# bf16 matmuls are also 1 cycle/row on PE and halve all DMA/SBUF traffic;
# the 2e-2 rel-err budget absorbs the 0.4% quantization easily.
BF = mybir.dt.bfloat16

SWAP16 = list(range(16, 32)) + list(range(16))  # pair-swap within quadrants


def _kernel_body(tc, io):
    nc = tc.nc
    xT, wqt, wkt, wvt, wot, cs, sn, mask4, o = (
        io["xT"], io["wqt"], io["wkt"], io["wvt"], io["wot"],
        io["cs"], io["sn"], io["mask4"], io["o"],
    )

    from contextlib import ExitStack

    ctx = ExitStack()
    const = ctx.enter_context(tc.tile_pool(name="const", bufs=1))
    xtp = ctx.enter_context(tc.tile_pool(name="xtp", bufs=2))
    work = ctx.enter_context(tc.tile_pool(name="work", bufs=6))
    expp = ctx.enter_context(tc.tile_pool(name="expp", bufs=10))
    outp = ctx.enter_context(tc.tile_pool(name="outp", bufs=8))
    psum = ctx.enter_context(tc.tile_pool(name="ps", bufs=2, space="PSUM"))

    # ---- x slice loads: two half DMAs per 512-wide slice (SP queue) so the
    # first projection matmuls can start before the whole slice lands ----
    def load_x(j, parts=2):
        jsl = slice(512 * j, 512 * (j + 1))
        xt = xtp.tile([128, DK, 512], BF, name="xt", tag="xt")
        src = xT[:, jsl].rearrange("(k p) t -> p k t", p=128)
        step = DK // parts
        for q in range(parts):
            nc.sync.dma_start(out=xt[:, q * step:(q + 1) * step, :],
                              in_=src[:, q * step:(q + 1) * step, :])
        return [xt[:, kt, :] for kt in range(DK)]

    xts0 = load_x(0)

    # ---- persistent SBUF tensors (weights batched, on the ACT DMA queue) ----
    w_all = {}
    # ACT-queue DMA issue order tracks first-use time: wq (first proj chunk,
    # halved so it starts early), wk, RoPE tables, then wv and wot (needed
    # only by the first out_proj, deep into attention(1)).
    def _wtile(src, nm):
        t_ = const.tile([128, DK, 256], BF, name=f"{nm}a", tag=f"{nm}a")
        s_ = src.rearrange("(k p) o -> p k o", p=128)
        if nm == "wq":
            nc.scalar.dma_start(out=t_[:, 0:DK // 2, :], in_=s_[:, 0:DK // 2, :])
            nc.scalar.dma_start(out=t_[:, DK // 2:DK, :], in_=s_[:, DK // 2:DK, :])
        else:
            nc.scalar.dma_start(out=t_, in_=s_)
        w_all[nm] = t_
    _wtile(wqt, "wq")
    _wtile(wkt, "wk")
    cs_sb = const.tile([128, T], BF, name="cs", tag="cs")
    nc.scalar.dma_start(out=cs_sb, in_=cs)
    sn_sb = const.tile([128, T], BF, name="sn", tag="sn")
    nc.scalar.dma_start(out=sn_sb, in_=sn)
    _wtile(wvt, "wv")
    # mask4 is consumed by attention(0)'s first diagonal step, well before
    # wot's first use (out_proj(0) inside attention(1)) -> load it earlier
    mask_sb = const.tile([128, 4, 512], BF, name="mask4", tag="mask4")
    nc.scalar.dma_start(out=mask_sb, in_=mask4.rearrange("r p f -> p r f"))
    wq_sb = [w_all["wq"][:, kt, :] for kt in range(DK)]
    wk_sb = [w_all["wk"][:, kt, :] for kt in range(DK)]
    wv_sb = [w_all["wv"][:, kt, :] for kt in range(DK)]
    wot_all = const.tile([128, 2, 1024], BF, name="wo", tag="wo")
    nc.scalar.dma_start(out=wot_all, in_=wot.rearrange("(k p) o -> p k o", p=128))
    wot_sb = [wot_all[:, m, :] for m in range(2)]
    onesf = const.tile([128, 8], F32, name="onesf", tag="onesf")
    nc.vector.memset(onesf, 1.0)

    qT_sb = [const.tile([128, T], BF, name=f"qT{m}", tag=f"qT{m}") for m in range(2)]
    kT_sb = [const.tile([128, T], BF, name=f"kT{m}", tag=f"kT{m}") for m in range(2)]
    at_sb = [const.tile([128, T], BF, name=f"at{m}", tag=f"at{m}") for m in range(2)]
    v_sb = [const.tile([128, HPC, HD + 1], BF, name=f"v{tt}", tag=f"v{tt}")
            for tt in range(NT)]
    def qk_chunk(j, xts, wsb, dstT, m):
        jsl = slice(512 * j, 512 * (j + 1))
        ps = psum.tile([128, 512], F32, name="ps_qk", tag="mm512")
        for kt in range(DK):
            nc.tensor.matmul(
                ps, wsb[kt][:, 128 * m:128 * (m + 1)], xts[kt],
                start=(kt == 0), stop=(kt == DK - 1),
            )
        sw = work.tile([128, 512], F32, name="sw", tag="sw")
        nc.vector.stream_shuffle(sw, ps, SWAP16)
        t1 = work.tile([128, 512], F32, name="t1", tag="t1")
        nc.gpsimd.tensor_mul(t1, sw, sn_sb[:, jsl])
        nc.vector.tensor_mul(dstT[m][:, jsl], ps, cs_sb[:, jsl])
        nc.gpsimd.tensor_add(dstT[m][:, jsl], dstT[m][:, jsl], t1)

    def v_chunk(j, xts, t4):
        tt = 4 * j + t4
        psw = psum.tile([128, 512], F32, name="ps_v", tag="mm512")
        ps = psw[:, 0:256]
        for kt in range(DK):
            nc.tensor.matmul(
                ps, xts[kt][:, 128 * t4:128 * (t4 + 1)], wv_sb[kt],
                start=(kt == 0), stop=(kt == DK - 1),
            )
        nc.scalar.activation(
            v_sb[tt][:, :, 0:HD], ps.rearrange("p (h d) -> p h d", h=HPC),
            mybir.ActivationFunctionType.Copy,
        )

    def proj_chunks(j, xts):
        for wsb, dstT in ((wq_sb, qT_sb), (wk_sb, kT_sb)):
            for m in range(2):
                yield lambda wsb=wsb, dstT=dstT, m=m: qk_chunk(j, xts, wsb, dstT, m)
        for t4 in range(4):
            yield lambda t4=t4: v_chunk(j, xts, t4)

    def out_chunk(j, t4, n, tail=False):
        t0 = 512 * j + 128 * t4
        tag = "pv" if (tail and (t4 + n) % 2) else "mm512"
        ops = psum.tile([128, 512], F32, name="ops", tag=tag)
        for m in range(2):
            nc.tensor.matmul(
                ops, at_sb[m][:, t0:t0 + 128],
                wot_sb[m][:, 512 * n:512 * (n + 1)],
                start=(m == 0), stop=(m == 1),
            )
        osb = outp.tile([128, 512], BF, name="osb", tag="osb")
        if tail and n == 1:
            # ACT is idle during the drain; splitting the copies across two
            # engines halves the output tail
            nc.scalar.activation(osb, ops, mybir.ActivationFunctionType.Copy)
        else:
            nc.vector.tensor_copy(osb, ops)
        nc.sync.dma_start(out=o[t0:t0 + 128, 512 * n:512 * (n + 1)], in_=osb)

    def out_chunks(j, tail=False):
        for t4 in range(4):
            for n in range(2):
                yield lambda t4=t4, n=n: out_chunk(j, t4, n, tail)

    def attention_steps(j, last=False):
        """One kt-step (both halves) per yield. PV emission trails the s/exp
        stage through a single queue that spans stream (m) boundaries, so PE
        always has s-matmul work in hand while ACT/Pool produce the masked
        exp tiles; each half's normalize is emitted the moment its final
        (kt==0) PV accumulation pops from the queue."""
        jsl = slice(512 * j, 512 * (j + 1))
        kmax = 4 * j + 3

        def normalize(m, pv, h):
            rz = work.tile([1, 512], R, name="rz", tag="rz")
            with nc.allow_low_precision(reason="f32r is bit-identical to f32"):
                # partition-shifted write (64->0): verified OK on HW
                nc.vector.reciprocal(rz, pv[64:65, :])
            rzb = work.tile([64, 512], R, name="rzb", tag="rzb")
            nc.gpsimd.partition_broadcast(rzb, rz)
            # partition-shifted DVE write (0:64 -> 64:128 for h==1): verified
            # OK on HW, saves the tmpB staging + SBUF-SBUF DMA
            nc.vector.tensor_mul(at_sb[m][64 * h:64 * (h + 1), jsl],
                                 pv[0:64, :], rzb)

        pend = []

        def pump():
            m, pvs, h, kt, e, first, last_kt = pend.pop(0)
            off = 512 - e.free_size()
            nc.tensor.matmul(pvs[h][0:HD + 1, off:512],
                             v_sb[kt][:, 2 * m + h, :],
                             e, start=first, stop=last_kt)
            if last_kt:
                normalize(m, pvs[h], h)

        # kt order: a full-width diagonal tile (r=0) opens the PSUM
        # accumulation, so every other diagonal tile (r=1..3) can shrink to
        # its live columns; off-diagonals follow descending.
        kt_order = [4 * j, 4 * j + 3, 4 * j + 2, 4 * j + 1] + \
            list(range(4 * j - 1, -1, -1))
        for m in range(2):
            pvs = [psum.tile([128, 512], F32, name=f"pv{h}", tag="pv")
                   for h in range(2)]
            for idx, kt in enumerate(kt_order):
                first, last_kt = idx == 0, idx == len(kt_order) - 1
                ksl = slice(128 * kt, 128 * (kt + 1))
                r = kt - 4 * j
                # Diagonal tiles r>=1: columns below 128*r are fully masked,
                # so s/exp/PV only cover [off:512] (bf16 matmul is 1 cyc/row
                # at any free size).
                off = 128 * r if r > 0 else 0
                # both halves share one 2-bank psum tile -> one wide exp
                s_ps = psum.tile([128, 2, 512], F32, name="s_ps", tag="s",
                                 bufs=2)
                for h in range(2):
                    hsl = slice(64 * h, 64 * (h + 1))
                    nc.tensor.matmul(s_ps[:, h, off:512], kT_sb[m][hsl, ksl],
                                     qT_sb[m][hsl, 512 * j + off:512 * (j + 1)],
                                     start=True, stop=True)
                e2 = expp.tile([128, 2, 512], BF, name="e2", tag="exp")
                nc.scalar.activation(e2[:, :, off:512], s_ps[:, :, off:512],
                                     mybir.ActivationFunctionType.Exp)
                if r >= 0:
                    # causal mask: multiply the diagonal 128-col triangle by a
                    # precomputed 0/1 bf16 mask (keep iff q_off >= 128r+k_off);
                    # columns below the triangle are skipped via `off`,
                    # columns above are always kept
                    w_ = 128 * (r + 1)
                    for h in range(2):
                        nc.vector.tensor_mul(
                            e2[:, h, off:w_], e2[:, h, off:w_],
                            mask_sb[:, r, off:w_])
                for h in range(2):
                    pend.append((m, pvs, h, kt, e2[:, h, off:512],
                                 first, last_kt))
                while len(pend) > 8:
                    pump()
                yield
        while pend:
            pump()
            if pend:
                pump()
            yield

    # ---- top-level software pipeline ----
    chunks0 = list(proj_chunks(0, xts0))
    for f in chunks0[:4]:
        f()
    # v-column ones land after the first qk chunks so the DVE queue reaches
    # RoPE(0) sooner
    for tt in range(NT):
        nc.vector.tensor_copy(
            v_sb[tt][:, :, HD:HD + 1],
            onesf[:, 0:HPC].rearrange("p (h x) -> p h x", x=1),
        )
    for f in chunks0[4:]:
        f()
    for j in range(NJ):
        fillers = []
        if j > 0:
            fillers += list(out_chunks(j - 1))
        if j + 1 < NJ:
            xts = load_x(j + 1)
            fillers += list(proj_chunks(j + 1, xts))
        steps = 2 * (4 * j + 4)
        period = max(1, -(-steps // (len(fillers) + 1)))
        i = 0
        for _ in attention_steps(j, last=(j == NJ - 1)):
            i += 1
            if i % period == 0 and fillers:
                fillers.pop(0)()
        for f in fillers:
            f()
    for f in out_chunks(NJ - 1, tail=True):
        f()

    ctx.close()


_NC_CACHE = None


def _build():
    global _NC_CACHE
    if _NC_CACHE is not None:
        return _NC_CACHE
    nc = bacc.Bacc("TRN2", target_bir_lowering=False, debug=False)
    io = {
        "xT": nc.dram_tensor("xT", [D, T], BF, kind="ExternalInput").ap(),
        "wqt": nc.dram_tensor("wqt", [D, 256], BF, kind="ExternalInput").ap(),
        "wkt": nc.dram_tensor("wkt", [D, 256], BF, kind="ExternalInput").ap(),
        "wvt": nc.dram_tensor("wvt", [D, 256], BF, kind="ExternalInput").ap(),
        "wot": nc.dram_tensor("wot", [256, D], BF, kind="ExternalInput").ap(),
        "cs": nc.dram_tensor("cs", [128, T], BF, kind="ExternalInput").ap(),
        "sn": nc.dram_tensor("sn", [128, T], BF, kind="ExternalInput").ap(),
        "mask4": nc.dram_tensor("mask4", [4, 128, 512], BF, kind="ExternalInput").ap(),
        "o": nc.dram_tensor("o", [T, D], BF, kind="ExternalOutput").ap(),
    }
    with tile.TileContext(nc) as tc:
        _kernel_body(tc, io)
    nc.compile()
    _NC_CACHE = nc
    return nc


def _host_inputs(x, wq, wk, wv, wo, freqs_cis, causal_mask):
    """Build the 8 per-core input maps (pure numpy preprocessing)."""
    x = np.asarray(x, dtype=np.float32)
    wq, wk, wv, wo = (np.asarray(a, dtype=np.float32) for a in (wq, wk, wv, wo))
    freqs_cis = np.asarray(freqs_cis, dtype=np.float32)

    # de-interleave head-dim pairs 16-wise so each RoPE partner sits in the
    # same 32-partition quadrant (stream_shuffle SWAP16 reaches it):
    # per head: [e0..e15, o0..o15, e16..e31, o16..o31]
    ph = np.concatenate([
        np.arange(0, 32, 2), np.arange(1, 32, 2),
        np.arange(32, 64, 2), np.arange(33, 64, 2),
    ])
    perm = np.concatenate([64 * h + ph for h in range(HPC)])

    cos_t = freqs_cis[:, :, 0].T  # (32, T)
    sin_t = freqs_cis[:, :, 1].T
    cs_head = np.concatenate([cos_t[0:16], cos_t[0:16], cos_t[16:32], cos_t[16:32]])
    sn_head = np.concatenate([-sin_t[0:16], sin_t[0:16], -sin_t[16:32], sin_t[16:32]])
    bf = ml_dtypes.bfloat16
    cs = np.tile(cs_head, (2, 1)).astype(bf)  # (128, T)
    sn = np.tile(sn_head, (2, 1)).astype(bf)
    # causal 0/1 masks per diagonal offset r: keep iff f >= 128*r + p
    pp = np.arange(128)[None, :, None]
    ff = np.arange(512)[None, None, :]
    rr = np.arange(4)[:, None, None]
    mask4 = (ff >= 128 * rr + pp).astype(bf)  # (4, 128, 512)

    scale = np.float32(HD ** -0.5)
    xT = [np.ascontiguousarray(x[b].T).astype(bf) for b in range(B)]
    in_maps = []
    for c in range(8):
        b, hg = c // HG, c % HG
        rows = slice(256 * hg, 256 * (hg + 1))
        wq_s = wq[rows][perm] * scale
        wk_s = wk[rows][perm]
        in_maps.append({
            "xT": xT[b],
            "wqt": np.ascontiguousarray(wq_s.T).astype(bf),
            "wkt": np.ascontiguousarray(wk_s.T).astype(bf),
            "wvt": np.ascontiguousarray(wv[rows].T).astype(bf),
            "wot": np.ascontiguousarray(wo[:, rows].T).astype(bf),
            "cs": cs,
            "sn": sn,
            "mask4": mask4,
        })
    return in_maps


def kernel(x, wq, wk, wv, wo, freqs_cis, causal_mask, _results_hook=None):
    nc = _build()
    in_maps = _host_inputs(x, wq, wk, wv, wo, freqs_cis, causal_mask)
    res = run_bass_kernel_spmd(nc, in_maps, core_ids=list(range(8)))
    if _results_hook is not None:
        _results_hook(res)
    parts = [r["o"].astype(np.float32) for r in res.results]
    out = np.stack([
        parts[0] + parts[1] + parts[2] + parts[3],
        parts[4] + parts[5] + parts[6] + parts[7],
    ])
    return out.astype(np.float32)



# revision 58
# speedup vs baseline: 1.0578x; 1.0578x over previous
"""Bass/Trainium2 kernel for GRAMAttention (B=2, T=2048, D=1024, H=16, hd=64).

Sharding: 8 cores = 2 batches (data parallel) x 4 head-groups of 4 heads
(tensor parallel: column-split wq/wk/wv, row-split wo). Each core computes a
partial (T, D) output for its batch; host sums the 4 head-group partials.

Per-core device pipeline (f32 data, f32r matmuls):
  host-transposed x^T (contiguous batched bf16 DMA) -> qT/kT projections
  into PSUM -> RoPE fused on DVE (stream_shuffle pair-swap + mul/mul/add
  reading PSUM directly) -> s^T = k @ q^T, both halves of a head pair into
  one 2-bank PSUM tile -> one wide exp on ACT per kt step -> causal mask
  post-exp via DVE multiply with precomputed 0/1 bf16 masks (diagonal tiles
  only) -> PV matmul with ones-column-augmented V (softmax denominator for
  free) -> normalize via DVE reciprocal (partition-shifted to lane 0) +
  Pool partition_broadcast -> row-split wo matmul.

Emission is software-pipelined at two levels: within an attention stream
pair the two halves' s-matmuls run one kt-step ahead of their PV
accumulations (so PE is never queued behind an exp it must wait for), and
PE-heavy filler chunks -- out_proj(j-1) and proj(j+1) -- are interleaved
between kt-steps because the attention phase is ACT(exp)-paced.
"""

import sys

if "/opt/trn_rl_repo" not in sys.path:
    sys.path.insert(0, "/opt/trn_rl_repo")

import ml_dtypes
import numpy as np

import concourse.bass as bass
import concourse.tile as tile
from concourse import bacc, mybir
from concourse.bass_utils import run_bass_kernel_spmd

B, T, D = 2, 2048, 1024
H = 16          # total heads
HPC = 4         # heads per core
HD = 64         # head dim
HG = H // HPC   # head groups (4) -> 8 cores = B * HG
DK = D // 128   # 8 contraction tiles for projections
NJ = T // 512   # 4 q/n slices
NT = T // 128   # 16 t tiles

F32 = mybir.dt.float32
# float32r: TF32-like relaxed fp32 matmul, 4x faster than fp32 at N>=256.
R = mybir.dt.float32r
# bf16 matmuls are also 1 cycle/row on PE and halve all DMA/SBUF traffic;
# the 2e-2 rel-err budget absorbs the 0.4% quantization easily.
BF = mybir.dt.bfloat16

SWAP16 = list(range(16, 32)) + list(range(16))  # pair-swap within quadrants


def _kernel_body(tc, io):
    nc = tc.nc
    xT, wqt, wkt, wvt, wot, cs, sn, mask4, o = (
        io["xT"], io["wqt"], io["wkt"], io["wvt"], io["wot"],
        io["cs"], io["sn"], io["mask4"], io["o"],
    )

    from contextlib import ExitStack

    ctx = ExitStack()
    const = ctx.enter_context(tc.tile_pool(name="const", bufs=1))
    # bufs=1: x(j+1)'s DMA then carries a WAR wait on proj(j)'s last read,
    # which keeps it from cutting in front of the head-phase weight/table
    # transfers on the shared DMA engines (SP's SEQ otherwise runs ahead and
    # issues it immediately)
    xtp = ctx.enter_context(tc.tile_pool(name="xtp", bufs=1))
    work = ctx.enter_context(tc.tile_pool(name="work", bufs=6))
    expp = ctx.enter_context(tc.tile_pool(name="expp", bufs=16))
    outp = ctx.enter_context(tc.tile_pool(name="outp", bufs=8))
    psum = ctx.enter_context(tc.tile_pool(name="ps", bufs=2, space="PSUM"))

    # ---- PE p-state warm-up: the cost model runs matmuls at half speed for
    # the first ~3us of PE activity. All real matmuls wait on the first DMAs
    # (~3us), so burn the ramp on junk matmuls against a memset tile. The
    # junk psum tile cycles through the "pv" tag, whose first real use is
    # far later (attention(0)).
    junk = const.tile([128, 128], BF, name="junk", tag="junk")
    nc.gpsimd.memset(junk, 0.0)
    jp = psum.tile([128, 512], F32, name="jp", tag="pv")
    for _ in range(28):
        nc.tensor.matmul(jp[:, 0:128], junk, junk, start=True, stop=True)

    # ---- x slice loads (SP queue), split so the first projection matmuls
    # can start before the whole slice lands ----
    def load_x(j, splits=(4, 4)):
        jsl = slice(512 * j, 512 * (j + 1))
        xt = xtp.tile([128, DK, 512], BF, name="xt", tag="xt")
        src = xT[:, jsl].rearrange("(k p) t -> p k t", p=128)
        q = 0
        for step in splits:
            nc.sync.dma_start(out=xt[:, q:q + step, :],
                              in_=src[:, q:q + step, :])
            q += step
        return [xt[:, kt, :] for kt in range(DK)]

    xts0 = load_x(0, splits=(1, 2, 2, 3))

    # ---- persistent SBUF tensors (weights batched, on the ACT DMA queue) ----
    w_all = {}
    # ACT-queue DMA issue order tracks first-use time: wq (first proj chunk,
    # halved so it starts early), wk, RoPE tables, then wv and wot (needed
    # only by the first out_proj, deep into attention(1)).
    def _wtile(src, nm, splits=(DK,)):
        t_ = const.tile([128, DK, 256], BF, name=f"{nm}a", tag=f"{nm}a")
        s_ = src.rearrange("(k p) o -> p k o", p=128)
        a = 0
        for step in splits:
            nc.scalar.dma_start(out=t_[:, a:a + step, :], in_=s_[:, a:a + step, :])
            a += step
        w_all[nm] = t_
    # DMA issue order tracks first-use: the input transfers serialize on the
    # shared DMA engines, so order = the head-phase schedule. First the wq
    # staircase (finer leading splits), then the j=0 slices of the RoPE
    # tables (RoPE(q,0) needs them before wk's first use), wk, the causal
    # mask (attention(0) first diag step), wv, the table remainders, wot.
    _wtile(wqt, "wq", splits=(1, 3, 4))
    cs_sb = const.tile([128, T], BF, name="cs", tag="cs")
    sn_sb = const.tile([128, T], BF, name="sn", tag="sn")
    nc.scalar.dma_start(out=cs_sb[:, 0:512], in_=cs[:, 0:512])
    nc.scalar.dma_start(out=sn_sb[:, 0:512], in_=sn[:, 0:512])
    _wtile(wkt, "wk")
    # The 128-wide diagonal strip mask (keep iff col >= partition) is the
    # same for every diagonal offset r; store it once, duplicated per head
    # half so one DVE multiply covers both halves.
    mask_sb = const.tile([128, 2, 128], BF, name="mask4", tag="mask4")
    nc.scalar.dma_start(out=mask_sb, in_=mask4.rearrange("p (h f) -> p h f", h=2))
    _wtile(wvt, "wv")
    nc.scalar.dma_start(out=cs_sb[:, 512:T], in_=cs[:, 512:T])
    nc.scalar.dma_start(out=sn_sb[:, 512:T], in_=sn[:, 512:T])
    wq_sb = [w_all["wq"][:, kt, :] for kt in range(DK)]
    wk_sb = [w_all["wk"][:, kt, :] for kt in range(DK)]
    wv_sb = [w_all["wv"][:, kt, :] for kt in range(DK)]
    wot_all = const.tile([128, 2, 1024], BF, name="wo", tag="wo")
    nc.scalar.dma_start(out=wot_all, in_=wot.rearrange("(k p) o -> p k o", p=128))
    wot_sb = [wot_all[:, m, :] for m in range(2)]


    qT_sb = [const.tile([128, T], BF, name=f"qT{m}", tag=f"qT{m}") for m in range(2)]
    kT_sb = [const.tile([128, T], BF, name=f"kT{m}", tag=f"kT{m}") for m in range(2)]
    at_sb = [const.tile([128, T], BF, name=f"at{m}", tag=f"at{m}") for m in range(2)]
    # v tiles carry 64 ones-columns after the HD value columns: the PV
    # matmul then writes the softmax denominator broadcast across psum rows
    # 64:128 for free (matmul cost depends only on the moving free size),
    # so normalize needs no partition_broadcast at all.
    v_sb = [const.tile([128, HPC, 2 * HD], BF, name=f"v{tt}", tag=f"v{tt}")
            for tt in range(NT)]
    def qk_chunk(j, xts, wsb, dstT, m):
        jsl = slice(512 * j, 512 * (j + 1))
        ps = psum.tile([128, 512], F32, name="ps_qk", tag="mm512")
        for kt in range(DK):
            nc.tensor.matmul(
                ps, wsb[kt][:, 128 * m:128 * (m + 1)], xts[kt],
                start=(kt == 0), stop=(kt == DK - 1),
            )
        # RoPE. The shuffle converts to bf16 so the sin-term multiply and
        # final add are SBUF-only bf16 ops: for j=0 they run on DVE (2x
        # mode, short latency -- attention(0) start depends on them); for
        # later slices, produced a full slice ahead, they go to the
        # otherwise-idle Pool engine (GPSIMD may not touch PSUM, but these
        # two are pure-SBUF). The PSUM-reading ops (shuffle, cos-mul) must
        # stay on DVE.
        # RoPE on DVE. One PSUM->bf16 copy up front (frees the psum bank
        # early), then the shuffle and the three elementwise ops all run as
        # SBUF bf16 in DVE 2x mode (cross-dtype shuffles fail the HW ISA
        # check, so the copy must precede the shuffle).
        tb = work.tile([128, 512], BF, name="tb", tag="tb")
        nc.vector.tensor_copy(tb, ps)
        sw = work.tile([128, 512], BF, name="sw", tag="sw")
        nc.vector.stream_shuffle(sw, tb, SWAP16)
        # sin-term mul and final add are pure-SBUF (legal on GPSIMD) and
        # off the critical path for j>0; attention(0)'s start waits on
        # RoPE(0), so that one stays on the faster DVE
        t1 = work.tile([128, 512], BF, name="t1", tag="t1")
        nc.gpsimd.tensor_mul(t1, sw, sn_sb[:, jsl])
        nc.vector.tensor_mul(dstT[m][:, jsl], tb, cs_sb[:, jsl])
        (nc.vector if j == 0 else nc.gpsimd).tensor_add(
            dstT[m][:, jsl], dstT[m][:, jsl], t1)

    def v_chunk(j, xts, t4):
        tt = 4 * j + t4
        psw = psum.tile([128, 512], F32, name="ps_v", tag="mm512")
        ps = psw[:, 0:256]
        for kt in range(DK):
            nc.tensor.matmul(
                ps, xts[kt][:, 128 * t4:128 * (t4 + 1)], wv_sb[kt],
                start=(kt == 0), stop=(kt == DK - 1),
            )
        nc.scalar.activation(
            v_sb[tt][:, :, 0:HD], ps.rearrange("p (h d) -> p h d", h=HPC),
            mybir.ActivationFunctionType.Copy,
        )

    def proj_chunks(j, xts):
        for wsb, dstT in ((wq_sb, qT_sb), (wk_sb, kT_sb)):
            for m in range(2):
                yield lambda wsb=wsb, dstT=dstT, m=m: qk_chunk(j, xts, wsb, dstT, m)
        for t4 in range(4):
            yield lambda t4=t4: v_chunk(j, xts, t4)

    def proj_chunks_lazy(j):
        # Issue the x DMAs as a filler step rather than at loop-top: issued
        # eagerly, the x(j+1) transfers cut in front of still-pending weight
        # and RoPE-table transfers on the shared DMA engines (arrival-order
        # arbitration) and starve the head of the pipeline.
        holder = {}

        def loadf():
            holder["x"] = load_x(j)

        yield loadf
        for wsb, dstT in ((wq_sb, qT_sb), (wk_sb, kT_sb)):
            for m in range(2):
                yield lambda wsb=wsb, dstT=dstT, m=m: qk_chunk(
                    j, holder["x"], wsb, dstT, m)
        for t4 in range(4):
            yield lambda t4=t4: v_chunk(j, holder["x"], t4)

    def out_chunk(j, t4, n):
        t0 = 512 * j + 128 * t4
        ops = psum.tile([128, 512], F32, name="ops", tag="mm512")
        for m in range(2):
            nc.tensor.matmul(
                ops, at_sb[m][:, t0:t0 + 128],
                wot_sb[m][:, 512 * n:512 * (n + 1)],
                start=(m == 0), stop=(m == 1),
            )
        osb = outp.tile([128, 512], BF, name="osb", tag="osb")
        nc.vector.tensor_copy(osb, ops)
        nc.sync.dma_start(out=o[t0:t0 + 128, 512 * n:512 * (n + 1)], in_=osb)

    def out_chunks(j):
        for t4 in range(4):
            for n in range(2):
                yield lambda t4=t4, n=n: out_chunk(j, t4, n)

    def out_chunk_pair(j, t4):
        # tail-only: attention is drained, so the wide "s" psum tiles are
        # free -- both n halves go into one 2-bank tile, one wide copy, one
        # wide DMA (fewer HWDGE slots: that device serializes the drain)
        t0 = 512 * j + 128 * t4
        ops = psum.tile([128, 2, 512], F32, name="opw", tag="s", bufs=2)
        # m-major: both m=0 matmuls are ready before the at[1] normalize
        # lands, so PE starts them during the drain
        for m in range(2):
            for n in range(2):
                nc.tensor.matmul(
                    ops[:, n, :], at_sb[m][:, t0:t0 + 128],
                    wot_sb[m][:, 512 * n:512 * (n + 1)],
                    start=(m == 0), stop=(m == 1),
                )
        osb = outp.tile([128, 1024], BF, name="osw", tag="osw")
        if t4 % 2:
            nc.scalar.copy(osb.rearrange("p (n f) -> p n f", n=2), ops)
        else:
            nc.vector.tensor_copy(osb.rearrange("p (n f) -> p n f", n=2), ops)
        q = [nc.sync, nc.scalar][t4 % 2]
        q.dma_start(out=o[t0:t0 + 128, :], in_=osb)

    def out_chunk_single(j, t4, n, ceng, q):
        t0 = 512 * j + 128 * t4
        ops = psum.tile([128, 512], F32, name="ops", tag="mm512")
        for m in range(2):
            nc.tensor.matmul(
                ops, at_sb[m][:, t0:t0 + 128],
                wot_sb[m][:, 512 * n:512 * (n + 1)],
                start=(m == 0), stop=(m == 1),
            )
        osb = outp.tile([128, 512], BF, name="osb", tag="osb")
        ceng(osb, ops)
        q.dma_start(out=o[t0:t0 + 128, 512 * n:512 * (n + 1)], in_=osb)

    def out_chunks_tail(j):
        for t4 in range(3):
            yield lambda t4=t4: out_chunk_pair(j, t4)
        # final 128 rows as two singles: copies land on ACT and DVE in
        # parallel and the last DMAs are half-size on separate queues
        yield lambda: out_chunk_single(j, 3, 0, nc.scalar.copy, nc.scalar)
        yield lambda: out_chunk_single(j, 3, 1, nc.vector.tensor_copy, nc.sync)

    def attention_steps(j, last=False):
        """One kt-step (both halves) per yield. PV emission trails the s/exp
        stage through a single queue that spans stream (m) boundaries, so PE
        always has s-matmul work in hand while ACT/Pool produce the masked
        exp tiles; each half's normalize is emitted the moment its final
        (kt==0) PV accumulation pops from the queue."""
        jsl = slice(512 * j, 512 * (j + 1))
        kmax = 4 * j + 3

        def normalize(m, pv, h, c0=0, c1=512):
            # pv rows 64:128 already hold the denominator broadcast across
            # 64 partitions (ones-columns in v), so normalize is just a
            # reciprocal + multiply, both on DVE -- no cross-engine hops
            rzb = work.tile([64, 512], R, name="rzb", tag="rzb")
            with nc.allow_low_precision(reason="f32r is bit-identical to f32"):
                nc.vector.reciprocal(rzb[:, c0:c1], pv[64:128, c0:c1])
            # partition-shifted DVE write (0:64 -> 64:128 for h==1): verified
            # OK on HW, saves the tmpB staging + SBUF-SBUF DMA
            nc.vector.tensor_mul(
                at_sb[m][64 * h:64 * (h + 1), 512 * j + c0:512 * j + c1],
                pv[0:64, c0:c1], rzb[:, c0:c1])

        pend = []

        def pump():
            m, pvs, h, kt, e, first, last_kt = pend.pop(0)
            off = 512 - e.free_size()
            nc.tensor.matmul(pvs[h][:, off:512],
                             v_sb[kt][:, 2 * m + h, :],
                             e, start=first, stop=last_kt)
            if last_kt:
                if last and m == 1:
                    # final stream: defer h0 and emit both halves'
                    # normalizes interleaved in 128-col chunks, so each
                    # trailing out-proj pair unblocks as its columns land
                    # instead of waiting on two full-width serial chains
                    if h == 1:
                        for c in range(2):
                            normalize(m, pvs[0], 0, 256 * c, 256 * (c + 1))
                            normalize(m, pvs[1], 1, 256 * c, 256 * (c + 1))
                else:
                    normalize(m, pvs[h], h)

        # kt order: a full-width diagonal tile (r=0) opens the PSUM
        # accumulation, so every other diagonal tile (r=1..3) can shrink to
        # its live columns. Off-diagonals next, shrinking diagonals LAST:
        # the next stream's first s-matmul waits on this stream's
        # second-to-last exp (s_ps double-buffer recycle), so ending on the
        # narrow tiles makes the stream hand-off cheap.
        kt_order = [4 * j] + list(range(4 * j - 1, -1, -1)) + \
            [4 * j + 1, 4 * j + 2, 4 * j + 3]
        for m in range(2):
            pvs = [psum.tile([128, 512], F32, name=f"pv{h}", tag="pv")
                   for h in range(2)]
            for idx, kt in enumerate(kt_order):
                first, last_kt = idx == 0, idx == len(kt_order) - 1
                ksl = slice(128 * kt, 128 * (kt + 1))
                r = kt - 4 * j
                # Diagonal tiles r>=1: columns below 128*r are fully masked,
                # so s/exp/PV only cover [off:512] (bf16 matmul is 1 cyc/row
                # at any free size).
                off = 128 * r if r > 0 else 0
                # both halves share one 2-bank psum tile -> one wide exp
                s_ps = psum.tile([128, 2, 512], F32, name="s_ps", tag="s",
                                 bufs=2)
                for h in range(2):
                    hsl = slice(64 * h, 64 * (h + 1))
                    nc.tensor.matmul(s_ps[:, h, off:512], kT_sb[m][hsl, ksl],
                                     qT_sb[m][hsl, 512 * j + off:512 * (j + 1)],
                                     start=True, stop=True)
                e2 = expp.tile([128, 2, 512], BF, name="e2", tag="exp")
                nc.scalar.activation(e2[:, :, off:512], s_ps[:, :, off:512],
                                     mybir.ActivationFunctionType.Exp)
                if r >= 0:
                    # causal mask: multiply the diagonal 128-col triangle by a
                    # precomputed 0/1 bf16 mask (keep iff q_off >= 128r+k_off);
                    # columns below the triangle are skipped via `off`,
                    # columns above are always kept. One mul covers both
                    # head halves (mask duplicated per h in SBUF).
                    w_ = 128 * (r + 1)
                    nc.vector.tensor_mul(
                        e2[:, :, off:w_], e2[:, :, off:w_], mask_sb)
                for h in range(2):
                    pend.append((m, pvs, h, kt, e2[:, h, off:512],
                                 first, last_kt))
                while len(pend) > 14:
                    pump()
                yield
        while pend:
            pump()
            if pend:
                pump()
            yield

    # ---- top-level software pipeline ----
    chunks0 = list(proj_chunks(0, xts0))
    for f in chunks0[:4]:
        f()
    # ones-columns for the denominator broadcast; Pool is idle in the
    # DMA-bound head window (SBUF-only memsets are legal on GPSIMD)
    for tt in range(NT):
        nc.gpsimd.memset(v_sb[tt][:, :, HD:2 * HD], 1.0)
    for f in chunks0[4:]:
        f()
    for j in range(NJ):
        fillers = []
        if j > 0:
            fillers += list(out_chunks(j - 1))
        if j + 1 < NJ:
            fillers += list(proj_chunks_lazy(j + 1))
        held = []
        if j == NJ - 1:
            # hold back a few out(j-1) chunks to flush after the loop: they
            # land between the final stream's normalize chain and the tail
            # out-proj matmuls, exactly where PE otherwise idles
            held = fillers[-3:]
            fillers = fillers[:-3]
        steps = 2 * (4 * j + 4)
        period = max(1, -(-steps // (len(fillers) + 1)))
        i = 0
        for _ in attention_steps(j, last=(j == NJ - 1)):
            i += 1
            if i % period == 0 and fillers:
                fillers.pop(0)()
        for f in fillers:
            f()
        for f in held:
            f()
    for f in out_chunks_tail(NJ - 1):
        f()

    ctx.close()


_NC_CACHE = None


def _build():
    global _NC_CACHE
    if _NC_CACHE is not None:
        return _NC_CACHE
    nc = bacc.Bacc("TRN2", target_bir_lowering=False, debug=False)
    io = {
        "xT": nc.dram_tensor("xT", [D, T], BF, kind="ExternalInput").ap(),
        "wqt": nc.dram_tensor("wqt", [D, 256], BF, kind="ExternalInput").ap(),
        "wkt": nc.dram_tensor("wkt", [D, 256], BF, kind="ExternalInput").ap(),
        "wvt": nc.dram_tensor("wvt", [D, 256], BF, kind="ExternalInput").ap(),
        "wot": nc.dram_tensor("wot", [256, D], BF, kind="ExternalInput").ap(),
        "cs": nc.dram_tensor("cs", [128, T], BF, kind="ExternalInput").ap(),
        "sn": nc.dram_tensor("sn", [128, T], BF, kind="ExternalInput").ap(),
        "mask4": nc.dram_tensor("mask4", [128, 256], BF, kind="ExternalInput").ap(),
        "o": nc.dram_tensor("o", [T, D], BF, kind="ExternalOutput").ap(),
    }
    with tile.TileContext(nc) as tc:
        _kernel_body(tc, io)
    nc.compile()
    _NC_CACHE = nc
    return nc


def _host_inputs(x, wq, wk, wv, wo, freqs_cis, causal_mask):
    """Build the 8 per-core input maps (pure numpy preprocessing)."""
    x = np.asarray(x, dtype=np.float32)
    wq, wk, wv, wo = (np.asarray(a, dtype=np.float32) for a in (wq, wk, wv, wo))
    freqs_cis = np.asarray(freqs_cis, dtype=np.float32)

    # de-interleave head-dim pairs 16-wise so each RoPE partner sits in the
    # same 32-partition quadrant (stream_shuffle SWAP16 reaches it):
    # per head: [e0..e15, o0..o15, e16..e31, o16..o31]
    ph = np.concatenate([
        np.arange(0, 32, 2), np.arange(1, 32, 2),
        np.arange(32, 64, 2), np.arange(33, 64, 2),
    ])
    perm = np.concatenate([64 * h + ph for h in range(HPC)])

    cos_t = freqs_cis[:, :, 0].T  # (32, T)
    sin_t = freqs_cis[:, :, 1].T
    cs_head = np.concatenate([cos_t[0:16], cos_t[0:16], cos_t[16:32], cos_t[16:32]])
    sn_head = np.concatenate([-sin_t[0:16], sin_t[0:16], -sin_t[16:32], sin_t[16:32]])
    bf = ml_dtypes.bfloat16
    cs = np.tile(cs_head, (2, 1)).astype(bf)  # (128, T)
    sn = np.tile(sn_head, (2, 1)).astype(bf)
    # causal 0/1 diagonal-strip mask: keep iff local col f >= partition p
    # (identical for every diagonal offset); duplicated for both head halves
    pp = np.arange(128)[:, None]
    ff = np.arange(128)[None, :]
    tri = (ff >= pp).astype(bf)
    mask4 = np.concatenate([tri, tri], axis=1)  # (128, 256)

    scale = np.float32(HD ** -0.5)
    xT = [np.ascontiguousarray(x[b].T).astype(bf) for b in range(B)]
    in_maps = []
    for c in range(8):
        b, hg = c // HG, c % HG
        rows = slice(256 * hg, 256 * (hg + 1))
        wq_s = wq[rows][perm] * scale
        wk_s = wk[rows][perm]
        in_maps.append({
            "xT": xT[b],
            "wqt": np.ascontiguousarray(wq_s.T).astype(bf),
            "wkt": np.ascontiguousarray(wk_s.T).astype(bf),
            "wvt": np.ascontiguousarray(wv[rows].T).astype(bf),
            "wot": np.ascontiguousarray(wo[:, rows].T).astype(bf),
            "cs": cs,
            "sn": sn,
            "mask4": mask4,
        })
    return in_maps


def kernel(x, wq, wk, wv, wo, freqs_cis, causal_mask, _results_hook=None):
    nc = _build()
    in_maps = _host_inputs(x, wq, wk, wv, wo, freqs_cis, causal_mask)
    res = run_bass_kernel_spmd(nc, in_maps, core_ids=list(range(8)))
    if _results_hook is not None:
        _results_hook(res)
    parts = [r["o"].astype(np.float32) for r in res.results]
    out = np.stack([
        parts[0] + parts[1] + parts[2] + parts[3],
        parts[4] + parts[5] + parts[6] + parts[7],
    ])
    return out.astype(np.float32)



# revision 66
# speedup vs baseline: 1.0793x; 1.0203x over previous
"""Bass/Trainium2 kernel for GRAMAttention (B=2, T=2048, D=1024, H=16, hd=64).

Sharding: 8 cores = 2 batches (data parallel) x 4 head-groups of 4 heads
(tensor parallel: column-split wq/wk/wv, row-split wo). Each core computes a
partial (T, D) output for its batch; host sums the 4 head-group partials.

Per-core device pipeline (f32 data, f32r matmuls):
  host-transposed x^T (contiguous batched bf16 DMA) -> qT/kT projections
  into PSUM -> RoPE fused on DVE (stream_shuffle pair-swap + mul/mul/add
  reading PSUM directly) -> s^T = k @ q^T, both halves of a head pair into
  one 2-bank PSUM tile -> one wide exp on ACT per kt step -> causal mask
  post-exp via DVE multiply with precomputed 0/1 bf16 masks (diagonal tiles
  only) -> PV matmul with ones-column-augmented V (softmax denominator for
  free) -> normalize via DVE reciprocal (partition-shifted to lane 0) +
  Pool partition_broadcast -> row-split wo matmul.

Emission is software-pipelined at two levels: within an attention stream
pair the two halves' s-matmuls run one kt-step ahead of their PV
accumulations (so PE is never queued behind an exp it must wait for), and
PE-heavy filler chunks -- out_proj(j-1) and proj(j+1) -- are interleaved
between kt-steps because the attention phase is ACT(exp)-paced.
"""

import sys

if "/opt/trn_rl_repo" not in sys.path:
    sys.path.insert(0, "/opt/trn_rl_repo")

import ml_dtypes
import numpy as np

import concourse.bass as bass
import concourse.tile as tile
from concourse import bacc, mybir
from concourse.bass_utils import run_bass_kernel_spmd

B, T, D = 2, 2048, 1024
H = 16          # total heads
HPC = 4         # heads per core
HD = 64         # head dim
HG = H // HPC   # head groups (4) -> 8 cores = B * HG
DK = D // 128   # 8 contraction tiles for projections
NJ = T // 512   # 4 q/n slices
NT = T // 128   # 16 t tiles

F32 = mybir.dt.float32
# float32r: TF32-like relaxed fp32 matmul, 4x faster than fp32 at N>=256.
R = mybir.dt.float32r
# bf16 matmuls are also 1 cycle/row on PE and halve all DMA/SBUF traffic;
# the 2e-2 rel-err budget absorbs the 0.4% quantization easily.
BF = mybir.dt.bfloat16

SWAP16 = list(range(16, 32)) + list(range(16))  # pair-swap within quadrants


def _kernel_body(tc, io):
    nc = tc.nc
    xT, wqt, wkt, wvt, wot, cs, sn, mask4, o = (
        io["xT"], io["wqt"], io["wkt"], io["wvt"], io["wot"],
        io["cs"], io["sn"], io["mask4"], io["o"],
    )

    from contextlib import ExitStack

    ctx = ExitStack()
    const = ctx.enter_context(tc.tile_pool(name="const", bufs=1))
    # bufs=1: x(j+1)'s DMA then carries a WAR wait on proj(j)'s last read,
    # which keeps it from cutting in front of the head-phase weight/table
    # transfers on the shared DMA engines (SP's SEQ otherwise runs ahead and
    # issues it immediately)
    xtp = ctx.enter_context(tc.tile_pool(name="xtp", bufs=1))
    work = ctx.enter_context(tc.tile_pool(name="work", bufs=6))
    expp = ctx.enter_context(tc.tile_pool(name="expp", bufs=16))
    outp = ctx.enter_context(tc.tile_pool(name="outp", bufs=8))
    psum = ctx.enter_context(tc.tile_pool(name="ps", bufs=2, space="PSUM"))

    # ---- PE p-state warm-up: the cost model runs matmuls at half speed for
    # the first ~3us of PE activity. All real matmuls wait on the first DMAs
    # (~3us), so burn the ramp on junk matmuls against a memset tile. The
    # junk psum tile cycles through the "pv" tag, whose first real use is
    # far later (attention(0)).
    junk = const.tile([128, 128], BF, name="junk", tag="junk")
    nc.gpsimd.memset(junk, 0.0)
    jp = psum.tile([128, 512], F32, name="jp", tag="pv")
    for _ in range(22):
        nc.tensor.matmul(jp[:, 0:128], junk, junk, start=True, stop=True)

    # ---- x slice loads (SP queue), split so the first projection matmuls
    # can start before the whole slice lands ----
    def load_x(j, splits=(4, 4), queues=None):
        jsl = slice(512 * j, 512 * (j + 1))
        xt = xtp.tile([128, DK, 512], BF, name="xt", tag="xt")
        src = xT[:, jsl].rearrange("(k p) t -> p k t", p=128)
        q = 0
        for i, step in enumerate(splits):
            eng = nc.sync if queues is None else queues[i]
            eng.dma_start(out=xt[:, q:q + step, :],
                          in_=src[:, q:q + step, :])
            q += step
        return [xt[:, kt, :] for kt in range(DK)]

    # head x load splits kt across the SP (HWDGE) and gpsimd (SWDGE) DMA
    # paths: SWDGE skips the HWDGE device, so the transfers run in parallel
    # and the weight DMAs on the scalar queue get earlier HWDGE slots
    xts0 = load_x(0, splits=(1, 2, 2, 3),
                  queues=(nc.sync, nc.gpsimd, nc.sync, nc.gpsimd))

    # ---- persistent SBUF tensors (weights batched, on the ACT DMA queue) ----
    w_all = {}
    # ACT-queue DMA issue order tracks first-use time: wq (first proj chunk,
    # halved so it starts early), wk, RoPE tables, then wv and wot (needed
    # only by the first out_proj, deep into attention(1)).
    def _wtile(src, nm, splits=(DK,)):
        t_ = const.tile([128, DK, 256], BF, name=f"{nm}a", tag=f"{nm}a")
        s_ = src.rearrange("(k p) o -> p k o", p=128)
        a = 0
        for step in splits:
            nc.scalar.dma_start(out=t_[:, a:a + step, :], in_=s_[:, a:a + step, :])
            a += step
        w_all[nm] = t_
    # DMA issue order tracks first-use: the input transfers serialize on the
    # shared DMA engines, so order = the head-phase schedule. First the wq
    # staircase (finer leading splits), then the j=0 slices of the RoPE
    # tables (RoPE(q,0) needs them before wk's first use), wk, the causal
    # mask (attention(0) first diag step), wv, the table remainders, wot.
    _wtile(wqt, "wq", splits=(1, 3, 4))
    cs_sb = const.tile([128, T], BF, name="cs", tag="cs")
    sn_sb = const.tile([128, T], BF, name="sn", tag="sn")
    nc.scalar.dma_start(out=cs_sb[:, 0:512], in_=cs[:, 0:512])
    nc.scalar.dma_start(out=sn_sb[:, 0:512], in_=sn[:, 0:512])
    _wtile(wkt, "wk")
    # The 128-wide diagonal strip mask (keep iff col >= partition) is the
    # same for every diagonal offset r; store it once, duplicated per head
    # half so one DVE multiply covers both halves.
    mask_sb = const.tile([128, 2, 128], BF, name="mask4", tag="mask4")
    nc.scalar.dma_start(out=mask_sb, in_=mask4.rearrange("p (h f) -> p h f", h=2))
    _wtile(wvt, "wv")
    nc.scalar.dma_start(out=cs_sb[:, 512:T], in_=cs[:, 512:T])
    nc.scalar.dma_start(out=sn_sb[:, 512:T], in_=sn[:, 512:T])
    wq_sb = [w_all["wq"][:, kt, :] for kt in range(DK)]
    wk_sb = [w_all["wk"][:, kt, :] for kt in range(DK)]
    wv_sb = [w_all["wv"][:, kt, :] for kt in range(DK)]
    wot_all = const.tile([128, 2, 1024], BF, name="wo", tag="wo")
    nc.scalar.dma_start(out=wot_all, in_=wot.rearrange("(k p) o -> p k o", p=128))
    wot_sb = [wot_all[:, m, :] for m in range(2)]


    qT_sb = [const.tile([128, T], BF, name=f"qT{m}", tag=f"qT{m}") for m in range(2)]
    kT_sb = [const.tile([128, T], BF, name=f"kT{m}", tag=f"kT{m}") for m in range(2)]
    at_sb = [const.tile([128, T], BF, name=f"at{m}", tag=f"at{m}") for m in range(2)]
    # v tiles carry 64 ones-columns after the HD value columns: the PV
    # matmul then writes the softmax denominator broadcast across psum rows
    # 64:128 for free (matmul cost depends only on the moving free size),
    # so normalize needs no partition_broadcast at all.
    v_sb = [const.tile([128, HPC, 2 * HD], BF, name=f"v{tt}", tag=f"v{tt}")
            for tt in range(NT)]
    def qk_chunk(j, xts, wsb, dstT, m):
        jsl = slice(512 * j, 512 * (j + 1))
        ps = psum.tile([128, 512], F32, name="ps_qk", tag="mm512")
        for kt in range(DK):
            nc.tensor.matmul(
                ps, wsb[kt][:, 128 * m:128 * (m + 1)], xts[kt],
                start=(kt == 0), stop=(kt == DK - 1),
            )
        # RoPE. The shuffle converts to bf16 so the sin-term multiply and
        # final add are SBUF-only bf16 ops: for j=0 they run on DVE (2x
        # mode, short latency -- attention(0) start depends on them); for
        # later slices, produced a full slice ahead, they go to the
        # otherwise-idle Pool engine (GPSIMD may not touch PSUM, but these
        # two are pure-SBUF). The PSUM-reading ops (shuffle, cos-mul) must
        # stay on DVE.
        # RoPE on DVE. One PSUM->bf16 copy up front (frees the psum bank
        # early), then the shuffle and the three elementwise ops all run as
        # SBUF bf16 in DVE 2x mode (cross-dtype shuffles fail the HW ISA
        # check, so the copy must precede the shuffle).
        tb = work.tile([128, 512], BF, name="tb", tag="tb")
        nc.vector.tensor_copy(tb, ps)
        sw = work.tile([128, 512], BF, name="sw", tag="sw")
        nc.vector.stream_shuffle(sw, tb, SWAP16)
        # sin-term mul and final add are pure-SBUF (legal on GPSIMD) and
        # off the critical path for j>0; attention(0)'s start waits on
        # RoPE(0), so that one stays on the faster DVE
        t1 = work.tile([128, 512], BF, name="t1", tag="t1")
        nc.gpsimd.tensor_mul(t1, sw, sn_sb[:, jsl])
        nc.vector.tensor_mul(dstT[m][:, jsl], tb, cs_sb[:, jsl])
        (nc.vector if j == 0 else nc.gpsimd).tensor_add(
            dstT[m][:, jsl], dstT[m][:, jsl], t1)

    def v_chunk(j, xts, t4):
        tt = 4 * j + t4
        psw = psum.tile([128, 512], F32, name="ps_v", tag="mm512")
        ps = psw[:, 0:256]
        for kt in range(DK):
            nc.tensor.matmul(
                ps, xts[kt][:, 128 * t4:128 * (t4 + 1)], wv_sb[kt],
                start=(kt == 0), stop=(kt == DK - 1),
            )
        nc.scalar.activation(
            v_sb[tt][:, :, 0:HD], ps.rearrange("p (h d) -> p h d", h=HPC),
            mybir.ActivationFunctionType.Copy,
        )

    def proj_chunks(j, xts):
        for wsb, dstT in ((wq_sb, qT_sb), (wk_sb, kT_sb)):
            for m in range(2):
                yield lambda wsb=wsb, dstT=dstT, m=m: qk_chunk(j, xts, wsb, dstT, m)
        for t4 in range(4):
            yield lambda t4=t4: v_chunk(j, xts, t4)

    def proj_chunks_lazy(j):
        # Issue the x DMAs as a filler step rather than at loop-top: issued
        # eagerly, the x(j+1) transfers cut in front of still-pending weight
        # and RoPE-table transfers on the shared DMA engines (arrival-order
        # arbitration) and starve the head of the pipeline.
        holder = {}

        def loadf():
            holder["x"] = load_x(j)

        yield loadf
        for wsb, dstT in ((wq_sb, qT_sb), (wk_sb, kT_sb)):
            for m in range(2):
                yield lambda wsb=wsb, dstT=dstT, m=m: qk_chunk(
                    j, holder["x"], wsb, dstT, m)
        for t4 in range(4):
            yield lambda t4=t4: v_chunk(j, holder["x"], t4)

    def out_chunk(j, t4, n):
        t0 = 512 * j + 128 * t4
        ops = psum.tile([128, 512], F32, name="ops", tag="mm512")
        for m in range(2):
            nc.tensor.matmul(
                ops, at_sb[m][:, t0:t0 + 128],
                wot_sb[m][:, 512 * n:512 * (n + 1)],
                start=(m == 0), stop=(m == 1),
            )
        osb = outp.tile([128, 512], BF, name="osb", tag="osb")
        nc.vector.tensor_copy(osb, ops)
        nc.sync.dma_start(out=o[t0:t0 + 128, 512 * n:512 * (n + 1)], in_=osb)

    def out_chunks(j):
        for t4 in range(4):
            for n in range(2):
                yield lambda t4=t4, n=n: out_chunk(j, t4, n)

    def out_chunk_pair(j, t4):
        # tail-only: attention is drained, so the wide "s" psum tiles are
        # free -- both n halves go into one 2-bank tile, one wide copy, one
        # wide DMA (fewer HWDGE slots: that device serializes the drain)
        t0 = 512 * j + 128 * t4
        ops = psum.tile([128, 2, 512], F32, name="opw", tag="s", bufs=2)
        # m-major: both m=0 matmuls are ready before the at[1] normalize
        # lands, so PE starts them during the drain
        for m in range(2):
            for n in range(2):
                nc.tensor.matmul(
                    ops[:, n, :], at_sb[m][:, t0:t0 + 128],
                    wot_sb[m][:, 512 * n:512 * (n + 1)],
                    start=(m == 0), stop=(m == 1),
                )
        osb = outp.tile([128, 1024], BF, name="osw", tag="osw")
        if t4 % 2:
            nc.scalar.copy(osb.rearrange("p (n f) -> p n f", n=2), ops)
        else:
            nc.vector.tensor_copy(osb.rearrange("p (n f) -> p n f", n=2), ops)
        q = [nc.sync, nc.scalar][t4 % 2]
        q.dma_start(out=o[t0:t0 + 128, :], in_=osb)

    def out_chunk_single(j, t4, n, ceng, q):
        t0 = 512 * j + 128 * t4
        ops = psum.tile([128, 512], F32, name="ops", tag="mm512")
        for m in range(2):
            nc.tensor.matmul(
                ops, at_sb[m][:, t0:t0 + 128],
                wot_sb[m][:, 512 * n:512 * (n + 1)],
                start=(m == 0), stop=(m == 1),
            )
        osb = outp.tile([128, 512], BF, name="osb", tag="osb")
        ceng(osb, ops)
        q.dma_start(out=o[t0:t0 + 128, 512 * n:512 * (n + 1)], in_=osb)

    def out_chunks_tail(j):
        for t4 in range(3):
            yield lambda t4=t4: out_chunk_pair(j, t4)
        # final 128 rows as two singles: copies land on ACT and DVE in
        # parallel and the last DMAs are half-size on separate queues
        yield lambda: out_chunk_single(j, 3, 0, nc.scalar.copy, nc.scalar)
        yield lambda: out_chunk_single(j, 3, 1, nc.vector.tensor_copy, nc.sync)

    def normalize(j, m, pv, h, c0=0, c1=512):
        # pv rows 64:128 already hold the denominator broadcast across
        # 64 partitions (ones-columns in v), so normalize is just a
        # reciprocal + multiply, both on DVE -- no cross-engine hops
        rzb = work.tile([64, 512], R, name="rzb", tag="rzb")
        with nc.allow_low_precision(reason="f32r is bit-identical to f32"):
            nc.vector.reciprocal(rzb[:, c0:c1], pv[64:128, c0:c1])
        # partition-shifted DVE write (0:64 -> 64:128 for h==1): verified
        # OK on HW, saves the tmpB staging + SBUF-SBUF DMA
        nc.vector.tensor_mul(
            at_sb[m][64 * h:64 * (h + 1), 512 * j + c0:512 * j + c1],
            pv[0:64, c0:c1], rzb[:, c0:c1])

    # PV emission trails the s/exp stage through one queue that persists
    # across stream AND slice boundaries: at a j hand-off the leftover PV
    # pumps give PE work while ACT restarts the exp pipeline for j+1.
    pend = []

    def pump():
        pv_h, mh, kt, e, first, last_kt, norm_fn = pend.pop(0)
        off = 512 - e.free_size()
        nc.tensor.matmul(pv_h[:, off:512], v_sb[kt][:, mh, :],
                         e, start=first, stop=last_kt)
        if last_kt and norm_fn is not None:
            norm_fn()

    def attention_steps(j, last=False):
        """One kt-step (both halves) per yield; each half's normalize is
        emitted the moment its final PV accumulation pops from the queue."""
        jsl = slice(512 * j, 512 * (j + 1))

        def make_norm(m, pvs, h):
            if last and m == 1:
                if h == 0:
                    return None
                # final stream: defer h0 and emit both halves' normalizes
                # interleaved in 256-col chunks, so each trailing out-proj
                # pair unblocks as its columns land instead of waiting on
                # two full-width serial chains
                def norm2():
                    for c in range(2):
                        normalize(j, m, pvs[0], 0, 256 * c, 256 * (c + 1))
                        normalize(j, m, pvs[1], 1, 256 * c, 256 * (c + 1))
                return norm2
            return lambda: normalize(j, m, pvs[h], h)

        # kt order: a full-width diagonal tile (r=0) opens the PSUM
        # accumulation, so every other diagonal tile (r=1..3) can shrink to
        # its live columns. Off-diagonals next, shrinking diagonals LAST:
        # the next stream's first s-matmul waits on this stream's
        # second-to-last exp (s_ps double-buffer recycle), so ending on the
        # narrow tiles makes the stream hand-off cheap.
        kt_order = [4 * j] + list(range(4 * j - 1, -1, -1)) + \
            [4 * j + 1, 4 * j + 2, 4 * j + 3]
        for m in range(2):
            pvs = [psum.tile([128, 512], F32, name=f"pv{h}", tag="pv")
                   for h in range(2)]
            for idx, kt in enumerate(kt_order):
                first, last_kt = idx == 0, idx == len(kt_order) - 1
                ksl = slice(128 * kt, 128 * (kt + 1))
                r = kt - 4 * j
                # Diagonal tiles r>=1: columns below 128*r are fully masked,
                # so s/exp/PV only cover [off:512] (bf16 matmul is 1 cyc/row
                # at any free size).
                off = 128 * r if r > 0 else 0
                # both halves share one 2-bank psum tile -> one wide exp
                s_ps = psum.tile([128, 2, 512], F32, name="s_ps", tag="s",
                                 bufs=2)
                for h in range(2):
                    hsl = slice(64 * h, 64 * (h + 1))
                    nc.tensor.matmul(s_ps[:, h, off:512], kT_sb[m][hsl, ksl],
                                     qT_sb[m][hsl, 512 * j + off:512 * (j + 1)],
                                     start=True, stop=True)
                e2 = expp.tile([128, 2, 512], BF, name="e2", tag="exp")
                nc.scalar.activation(e2[:, :, off:512], s_ps[:, :, off:512],
                                     mybir.ActivationFunctionType.Exp)
                if r >= 0:
                    # causal mask: multiply the diagonal 128-col triangle by a
                    # precomputed 0/1 bf16 mask (keep iff q_off >= 128r+k_off);
                    # columns below the triangle are skipped via `off`,
                    # columns above are always kept. One mul covers both
                    # head halves (mask duplicated per h in SBUF).
                    w_ = 128 * (r + 1)
                    nc.vector.tensor_mul(
                        e2[:, :, off:w_], e2[:, :, off:w_], mask_sb)
                for h in range(2):
                    pend.append((pvs[h], 2 * m + h, kt, e2[:, h, off:512],
                                 first, last_kt, make_norm(m, pvs, h)))
                while len(pend) > 14:
                    pump()
                yield
        if last:
            while pend:
                pump()
                if pend:
                    pump()
                yield

    # ---- top-level software pipeline ----
    chunks0 = list(proj_chunks(0, xts0))
    for f in chunks0[:4]:
        f()
    # ones-columns for the denominator broadcast; Pool is idle in the
    # DMA-bound head window (SBUF-only memsets are legal on GPSIMD)
    for tt in range(NT):
        nc.gpsimd.memset(v_sb[tt][:, :, HD:2 * HD], 1.0)
    for f in chunks0[4:]:
        f()
    for j in range(NJ):
        # all out-proj chunks run as fillers of the LAST attention phase:
        # the exp workload grows with j (causal) while the proj filler
        # supply ends at proj(NJ-1), so attention(NJ-1) is the only
        # exp-bound phase -- park every deferrable PE chunk there
        fillers = []
        if j == NJ - 1:
            for jj in range(NJ - 1):
                fillers += list(out_chunks(jj))
        if j + 1 < NJ:
            fillers += list(proj_chunks_lazy(j + 1))
        held = []
        if j == NJ - 1:
            # hold back a few out chunks to flush after the loop: they land
            # between the final stream's normalize chain and the tail
            # out-proj matmuls, exactly where PE otherwise idles
            held = fillers[-3:]
            fillers = fillers[:-3]
        steps = 2 * (4 * j + 4)
        period = max(1, -(-steps // (len(fillers) + 1)))
        i = 0
        for _ in attention_steps(j, last=(j == NJ - 1)):
            i += 1
            if i % period == 0 and fillers:
                fillers.pop(0)()
        for f in fillers:
            f()
        for f in held:
            f()
    for f in out_chunks_tail(NJ - 1):
        f()

    ctx.close()


_NC_CACHE = None


def _build():
    global _NC_CACHE
    if _NC_CACHE is not None:
        return _NC_CACHE
    nc = bacc.Bacc("TRN2", target_bir_lowering=False, debug=False)
    io = {
        "xT": nc.dram_tensor("xT", [D, T], BF, kind="ExternalInput").ap(),
        "wqt": nc.dram_tensor("wqt", [D, 256], BF, kind="ExternalInput").ap(),
        "wkt": nc.dram_tensor("wkt", [D, 256], BF, kind="ExternalInput").ap(),
        "wvt": nc.dram_tensor("wvt", [D, 256], BF, kind="ExternalInput").ap(),
        "wot": nc.dram_tensor("wot", [256, D], BF, kind="ExternalInput").ap(),
        "cs": nc.dram_tensor("cs", [128, T], BF, kind="ExternalInput").ap(),
        "sn": nc.dram_tensor("sn", [128, T], BF, kind="ExternalInput").ap(),
        "mask4": nc.dram_tensor("mask4", [128, 256], BF, kind="ExternalInput").ap(),
        "o": nc.dram_tensor("o", [T, D], BF, kind="ExternalOutput").ap(),
    }
    with tile.TileContext(nc) as tc:
        _kernel_body(tc, io)
    nc.compile()
    _NC_CACHE = nc
    return nc


def _host_inputs(x, wq, wk, wv, wo, freqs_cis, causal_mask):
    """Build the 8 per-core input maps (pure numpy preprocessing)."""
    x = np.asarray(x, dtype=np.float32)
    wq, wk, wv, wo = (np.asarray(a, dtype=np.float32) for a in (wq, wk, wv, wo))
    freqs_cis = np.asarray(freqs_cis, dtype=np.float32)

    # de-interleave head-dim pairs 16-wise so each RoPE partner sits in the
    # same 32-partition quadrant (stream_shuffle SWAP16 reaches it):
    # per head: [e0..e15, o0..o15, e16..e31, o16..o31]
    ph = np.concatenate([
        np.arange(0, 32, 2), np.arange(1, 32, 2),
        np.arange(32, 64, 2), np.arange(33, 64, 2),
    ])
    perm = np.concatenate([64 * h + ph for h in range(HPC)])

    cos_t = freqs_cis[:, :, 0].T  # (32, T)
    sin_t = freqs_cis[:, :, 1].T
    cs_head = np.concatenate([cos_t[0:16], cos_t[0:16], cos_t[16:32], cos_t[16:32]])
    sn_head = np.concatenate([-sin_t[0:16], sin_t[0:16], -sin_t[16:32], sin_t[16:32]])
    bf = ml_dtypes.bfloat16
    cs = np.tile(cs_head, (2, 1)).astype(bf)  # (128, T)
    sn = np.tile(sn_head, (2, 1)).astype(bf)
    # causal 0/1 diagonal-strip mask: keep iff local col f >= partition p
    # (identical for every diagonal offset); duplicated for both head halves
    pp = np.arange(128)[:, None]
    ff = np.arange(128)[None, :]
    tri = (ff >= pp).astype(bf)
    mask4 = np.concatenate([tri, tri], axis=1)  # (128, 256)

    scale = np.float32(HD ** -0.5)
    xT = [np.ascontiguousarray(x[b].T).astype(bf) for b in range(B)]
    in_maps = []
    for c in range(8):
        b, hg = c // HG, c % HG
        rows = slice(256 * hg, 256 * (hg + 1))
        wq_s = wq[rows][perm] * scale
        wk_s = wk[rows][perm]
        in_maps.append({
            "xT": xT[b],
            "wqt": np.ascontiguousarray(wq_s.T).astype(bf),
            "wkt": np.ascontiguousarray(wk_s.T).astype(bf),
            "wvt": np.ascontiguousarray(wv[rows].T).astype(bf),
            "wot": np.ascontiguousarray(wo[:, rows].T).astype(bf),
            "cs": cs,
            "sn": sn,
            "mask4": mask4,
        })
    return in_maps


def kernel(x, wq, wk, wv, wo, freqs_cis, causal_mask, _results_hook=None):
    nc = _build()
    in_maps = _host_inputs(x, wq, wk, wv, wo, freqs_cis, causal_mask)
    res = run_bass_kernel_spmd(nc, in_maps, core_ids=list(range(8)))
    if _results_hook is not None:
        _results_hook(res)
    parts = [r["o"].astype(np.float32) for r in res.results]
    out = np.stack([
        parts[0] + parts[1] + parts[2] + parts[3],
        parts[4] + parts[5] + parts[6] + parts[7],
    ])
    return out.astype(np.float32)

